# revision 27
# baseline (speedup 1.0000x reference)
"""Distributed Trainium2 Bass kernel for the 1x1-conv multi-head attention block.

Reference computation (per batch b of 4):
    qkv = w_qkv @ x            x: [256, 4096] (channels x spatial), w_qkv: [384, 256]
    q,k,v = split(qkv); per head h (4 heads, d=32): q *= d**-0.5
    sim = q^T k;  attn = softmax(sim, axis=k);  o = attn @ v^T
    y = w_out @ o + b_out      w_out: [256, 128]

Distribution: 8 cores = 4 batches x 2 query-halves. Each core computes
q/k/v projections for its batch (k, v for all 4096 positions; q only for
its 2048 query positions), full attention for its query half across all
4 heads, and the output projection for its rows. No cross-core
reduction is needed; the host concatenates the 8 disjoint output blocks.

On-chip layout ("simT orientation"): scores are computed transposed,
[k_pos (partitions), q_pos (free)], so softmax denominators come from a
ones-vector matmul (partition-dim reduction on the PE) and the attn@v
contraction consumes the exp'd scores directly as the moving operand —
no transposes in the inner loop. exp() has no max-subtraction: scores
are ~N(0,1) so exp is safely in range. All matmuls run in bf16 (PSUM
accumulation is fp32); softmax sums/normalization in fp32.
"""

import sys

if "/opt/trn_rl_repo" not in sys.path:
    sys.path.insert(0, "/opt/trn_rl_repo")

import numpy as np
import ml_dtypes

import concourse.bass as bass
import concourse.mybir as mybir
import concourse.tile as tile
from concourse import bacc

BF16 = mybir.dt.bfloat16
F32 = mybir.dt.float32

N_CORES = 8
HEADS = 4
DIM_HEAD = 32
SCALE = DIM_HEAD ** -0.5


class Cfg:
    def __init__(self, seq=4096, q_half=2048, q_tile=512, dim=256):
        self.seq = seq                  # total k/v positions
        self.q_half = q_half            # q positions handled by this core
        self.q_tile = q_tile            # q columns per PSUM tile (<=512)
        self.dim = dim                  # input channels
        self.hidden = HEADS * DIM_HEAD  # 128
        self.n_ktiles = seq // 128
        self.n_qtiles = q_half // q_tile
        self.n_stiles = seq // q_tile   # spatial tiles for k projection
        self.n_ctiles = dim // 128      # contraction tiles for projections


FULL = Cfg()


def build_nc(cfg=FULL, n_cores=N_CORES, dbg=False):
    nc = bacc.Bacc("TRN2", target_bir_lowering=False, debug=False,
                   num_devices=n_cores)

    # ---- DRAM I/O ----------------------------------------------------
    x_d = nc.dram_tensor("x", [cfg.n_ctiles, 128, cfg.seq], BF16,
                         kind="ExternalInput")
    xq_d = nc.dram_tensor("xq", [cfg.n_ctiles, 128, cfg.q_half], BF16,
                          kind="ExternalInput")
    wq_d = nc.dram_tensor("wq", [cfg.n_ctiles, 128, 128], BF16,
                          kind="ExternalInput")
    wk_d = nc.dram_tensor("wk", [cfg.n_ctiles, 128, 128], BF16,
                          kind="ExternalInput")
    wv_d = nc.dram_tensor("wv", [cfg.n_ctiles, 128, 128], BF16,
                          kind="ExternalInput")
    wo_d = nc.dram_tensor("wo", [128, cfg.dim], BF16, kind="ExternalInput")
    bo_d = nc.dram_tensor("bo", [cfg.n_ctiles, 128, 1], F32, kind="ExternalInput")
    ones_d = nc.dram_tensor("ones", [128, 1], BF16, kind="ExternalInput")
    ind_d = nc.dram_tensor("ind", [HEADS, 128], F32, kind="ExternalInput")
    y_d = nc.dram_tensor("out", [cfg.n_ctiles, 128, cfg.q_half], F32,
                         kind="ExternalOutput")

    dbg_d = None
    if dbg:
        dbg_d = {
            "dbg_qT": nc.dram_tensor("dbg_qT", [128, cfg.q_half], BF16,
                                     kind="ExternalOutput"),
            "dbg_kT": nc.dram_tensor("dbg_kT", [128, cfg.seq], BF16,
                                     kind="ExternalOutput"),
            "dbg_v": nc.dram_tensor("dbg_v", [128, cfg.n_ktiles, HEADS, 33],
                                    BF16, kind="ExternalOutput"),
            "dbg_e": nc.dram_tensor("dbg_e", [128, 2, cfg.q_tile], BF16,
                                    kind="ExternalOutput"),
            "dbg_s4": nc.dram_tensor("dbg_s4", [1, HEADS, cfg.q_tile], F32,
                                     kind="ExternalOutput"),
            "dbg_bc": nc.dram_tensor("dbg_bc", [128, cfg.q_tile], F32,
                                     kind="ExternalOutput"),
            "dbg_outn": nc.dram_tensor("dbg_outn", [128, cfg.q_half], BF16,
                                       kind="ExternalOutput"),
        }

    with tile.TileContext(nc) as tc:
        _kernel_body(tc, cfg, x_d, xq_d, wq_d, wk_d, wv_d, wo_d, bo_d,
                     ones_d, ind_d, y_d, dbg_d)
    nc.compile()
    return nc


def _kernel_body(tc, cfg, x_d, xq_d, wq_d, wk_d, wv_d, wo_d, bo_d,
                 ones_d, ind_d, y_d, dbg_d=None):
    nc = tc.nc
    NK, NQ, QT = cfg.n_ktiles, cfg.n_qtiles, cfg.q_tile
    NC_, NS = cfg.n_ctiles, cfg.n_stiles

    from contextlib import ExitStack
    with ExitStack() as ctx:
        ep = ctx.enter_context

        consts = ep(tc.tile_pool(name="consts", bufs=1))
        persist = ep(tc.tile_pool(name="persist", bufs=1))

        # ---- load inputs & weights ----------------------------------
        x_sb = persist.tile([128, NC_, cfg.seq], BF16, tag="x")
        xq_sb = persist.tile([128, NC_, cfg.q_half], BF16, tag="xq")
        wq_sb = consts.tile([128, NC_, 128], BF16, tag="wq")
        wk_sb = consts.tile([128, NC_, 128], BF16, tag="wk")
        wv_sb = consts.tile([128, NC_, 128], BF16, tag="wv")
        wo_sb = consts.tile([128, cfg.dim], BF16, tag="wo")
        bo_sb = consts.tile([128, NC_], F32, tag="bo")
        ones_sb = consts.tile([128, 1], BF16, tag="ones")
        ind_sb = consts.tile([HEADS, 128], F32, tag="ind")

        # Prime the ACT exp table-set before the PE-dense phases: the
        # one-time ~2.7us ACT_TABLE_LOAD otherwise lands on the first
        # real exp, idles the PE long enough for HAM to re-throttle the
        # clock to 1.2 GHz, and it never recovers mid-kernel.
        prime = consts.tile([128, 8], F32, tag="prime")
        nc.vector.memset(prime[:], 0.0)
        nc.scalar.activation(prime[:], prime[:],
                             mybir.ActivationFunctionType.Exp)

        for c in range(NC_):
            nc.sync.dma_start(x_sb[:, c, :], x_d[c])
            nc.sync.dma_start(xq_sb[:, c, :], xq_d[c])
            nc.sync.dma_start(wq_sb[:, c, :], wq_d[c])
            nc.sync.dma_start(wk_sb[:, c, :], wk_d[c])
            nc.sync.dma_start(wv_sb[:, c, :], wv_d[c])
            nc.sync.dma_start(bo_sb[:, c : c + 1], bo_d[c])
        nc.sync.dma_start(wo_sb[:], wo_d[:])
        nc.sync.dma_start(ones_sb[:], ones_d[:])
        nc.sync.dma_start(ind_sb[:], ind_d[:])

        # ---- projections --------------------------------------------
        # qT [d*heads=128, q_half], kT [128, seq] (channel-major)
        qT_sb = persist.tile([128, cfg.q_half], BF16, tag="qT")
        kT_sb = persist.tile([128, cfg.seq], BF16, tag="kT")
        # v spatial-major, augmented with a ones column per head:
        # v_sb[p, t, h, 0:32] = v[t*128+p, h*32+d]; v_sb[p, t, h, 32] = 1
        v_sb = persist.tile([128, NK, HEADS, 33], BF16, tag="v")

        with tc.tile_pool(name="proj_ps", bufs=3, space="PSUM") as proj_ps, \
             tc.tile_pool(name="projv_ps", bufs=3, space="PSUM") as projv_ps:
            # q projection: lhsT = wq tile [128c, 128o], rhs = xq [128c, QT]
            for s in range(cfg.q_half // QT):
                pt = proj_ps.tile([128, QT], F32, tag="proj")
                for c in range(NC_):
                    nc.tensor.matmul(pt[:], wq_sb[:, c, :],
                                     xq_sb[:, c, s * QT : (s + 1) * QT],
                                     start=(c == 0), stop=(c == NC_ - 1))
                nc.vector.tensor_copy(qT_sb[:, s * QT : (s + 1) * QT], pt[:])
            # k projection over full seq
            for s in range(NS):
                pt = proj_ps.tile([128, QT], F32, tag="proj")
                for c in range(NC_):
                    nc.tensor.matmul(pt[:], wk_sb[:, c, :],
                                     x_sb[:, c, s * QT : (s + 1) * QT],
                                     start=(c == 0), stop=(c == NC_ - 1))
                nc.vector.tensor_copy(kT_sb[:, s * QT : (s + 1) * QT], pt[:])
            # v projection, spatial-major: lhsT = x tile [128c, 128s],
            # rhs = wv [128c, 128o] -> out [128s, 128o]
            nc.vector.memset(v_sb[:, :, :, 32], 1.0)
            for t in range(NK):
                pt = projv_ps.tile([128, 128], F32, tag="projv")
                for c in range(NC_):
                    nc.tensor.matmul(pt[:],
                                     x_sb[:, c, t * 128 : (t + 1) * 128],
                                     wv_sb[:, c, :],
                                     start=(c == 0), stop=(c == NC_ - 1))
                for h in range(HEADS):
                    nc.vector.tensor_copy(v_sb[:, t, h, 0:32],
                                          pt[:, 32 * h : 32 * h + 32])

        # ---- attention ----------------------------------------------
        # per q-tile: accumulate outT [128 (h*32+d), QT] and per-head
        # sums [row 32h, QT] over all k tiles.
        attn_ctx = ExitStack()
        with attn_ctx:
            sim_ps = attn_ctx.enter_context(
                tc.tile_pool(name="sim_ps", bufs=2, space="PSUM"))
            acc_ps = attn_ctx.enter_context(
                tc.tile_pool(name="acc_ps", bufs=1, space="PSUM"))
            e_pool = attn_ctx.enter_context(
                tc.tile_pool(name="e_pool", bufs=3))
            norm_pool = attn_ctx.enter_context(
                tc.tile_pool(name="norm", bufs=2))

            outn_sb = persist.tile([128, cfg.q_half], BF16, tag="outn")

            self_attention(nc, cfg, tc, sim_ps, acc_ps, e_pool,
                           norm_pool, qT_sb, kT_sb, v_sb,
                           outn_sb, dbg_d)

        if dbg_d is not None:
            nc.sync.dma_start(dbg_d["dbg_qT"][:], qT_sb[:])
            nc.sync.dma_start(dbg_d["dbg_kT"][:], kT_sb[:])
            nc.sync.dma_start(dbg_d["dbg_v"][:], v_sb[:])
            nc.sync.dma_start(dbg_d["dbg_outn"][:], outn_sb[:])

        # ---- output projection --------------------------------------
        with tc.tile_pool(name="y", bufs=4) as y_pool, \
             tc.tile_pool(name="y_ps", bufs=4, space="PSUM") as yp_ps:
            for c in range(NC_):
                for s in range(cfg.q_half // QT):
                    pt = yp_ps.tile([128, QT], F32, tag="yp")
                    nc.tensor.matmul(pt[:], wo_sb[:, c * 128 : (c + 1) * 128],
                                     outn_sb[:, s * QT : (s + 1) * QT],
                                     start=True, stop=True)
                    yt = y_pool.tile([128, QT], F32, tag="yt")
                    nc.vector.tensor_scalar_add(yt[:], pt[:],
                                                bo_sb[:, c : c + 1])
                    nc.sync.dma_start(y_d[c, :, s * QT : (s + 1) * QT], yt[:])


def self_attention(nc, cfg, tc, sim_ps, acc_ps, e_pool, norm_pool,
                   qT_sb, kT_sb, v_sb, outn_sb, dbg_d=None):
        NK, NQ, QT = cfg.n_ktiles, cfg.n_qtiles, cfg.q_tile
        for qi in range(NQ):
            qs = slice(qi * QT, (qi + 1) * QT)
            accs = [acc_ps.tile([128, QT], F32, tag=f"acc{h}",
                                name=f"acc{h}_{qi}")
                    for h in range(HEADS)]

            for t in range(NK):
                # scores^T for 2+2 heads into two 2-bank psum tiles
                for pair in range(2):
                    sim = sim_ps.tile([128, 2, QT], F32, tag="sim")
                    for j in range(2):
                        h = pair * 2 + j
                        nc.tensor.matmul(
                            sim[:, j, :],
                            kT_sb[32 * h : 32 * h + 32, t * 128 : (t + 1) * 128],
                            qT_sb[32 * h : 32 * h + 32, qs],
                            start=True, stop=True,
                            tile_position=(32 * h, 0),
                        )
                    e_sb = e_pool.tile([128, 2, QT], BF16, tag="e")
                    nc.scalar.activation(e_sb[:], sim[:],
                                         mybir.ActivationFunctionType.Exp)
                    if dbg_d is not None and qi == 0 and t == 0 and pair == 0:
                        nc.sync.dma_start(dbg_d["dbg_e"][:], e_sb[:])
                    for j in range(2):
                        h = pair * 2 + j
                        # attn@v (+ denominator): lhsT = [v_h | 1] chunk
                        # [128 kpos, 33]; row 32 of the output accumulates
                        # sum(E) per q column.
                        nc.tensor.matmul(
                            accs[h][0:33, :],
                            v_sb[:, t, h, :],
                            e_sb[:, j, :],
                            start=(t == 0), stop=(t == NK - 1),
                            skip_group_check=True,
                        )

            # epilogue: gather the 4 sum rows onto partition 0, recip,
            # broadcast each head's row across its 32 partitions, then
            # normalize.
            s4 = norm_pool.tile([1, HEADS, QT], F32, tag="s4")
            for h in range(HEADS):
                nc.vector.tensor_copy(s4[0:1, h, :], accs[h][32:33, :])
            if dbg_d is not None and qi == 0:
                nc.sync.dma_start(dbg_d["dbg_s4"][:], s4[:])
            r4 = norm_pool.tile([1, HEADS, QT], F32, tag="r4")
            nc.vector.reciprocal(r4[:], s4[:])
            bcs = [norm_pool.tile([32, QT], F32, tag=f"bc{h}",
                                  name=f"bc{h}_{qi}")
                   for h in range(HEADS)]
            for h in range(HEADS):
                nc.gpsimd.partition_broadcast(bcs[h][:], r4[0:1, h, :])
            if dbg_d is not None and qi == 0:
                for h in range(HEADS):
                    nc.sync.dma_start(dbg_d["dbg_bc"][32 * h : 32 * h + 32, :],
                                      bcs[h][:])
            for h in range(HEADS):
                nc.vector.tensor_mul(outn_sb[32 * h : 32 * h + 32, qs],
                                     accs[h][0:32, :], bcs[h][:])


# ---------------------------------------------------------------------
# host side
# ---------------------------------------------------------------------

def make_in_maps(x, w_qkv, w_out, b_out, cfg=FULL, n_cores=N_CORES):
    """Shard the full inputs into per-core input maps."""
    b, dim, H, W = x.shape
    seq = H * W
    bf = ml_dtypes.bfloat16

    wq = (w_qkv[0:128] * SCALE).astype(np.float32)
    wk = w_qkv[128:256]
    wv = w_qkv[256:384]
    # lhsT layouts [ctile, 128c, 128o]
    wq_t = np.ascontiguousarray(
        wq.T.reshape(cfg.n_ctiles, 128, 128)).astype(bf)
    wk_t = np.ascontiguousarray(
        wk.T.reshape(cfg.n_ctiles, 128, 128)).astype(bf)
    wv_t = np.ascontiguousarray(
        wv.T.reshape(cfg.n_ctiles, 128, 128)).astype(bf)
    wo_t = np.ascontiguousarray(w_out.T).astype(bf)          # [128, 256]
    bo = b_out.reshape(cfg.n_ctiles, 128, 1).astype(np.float32)
    ones = np.ones((128, 1), dtype=bf)
    ind = np.zeros((HEADS, 128), dtype=np.float32)
    for h in range(HEADS):
        ind[h, 32 * h : 32 * h + 32] = 1.0

    in_maps = []
    for core in range(n_cores):
        bi, half = core // 2, core % 2
        xb = x[bi].reshape(dim, seq)
        x_bf = xb.reshape(cfg.n_ctiles, 128, seq).astype(bf)
        xq_bf = np.ascontiguousarray(
            xb[:, half * cfg.q_half : (half + 1) * cfg.q_half]
        ).reshape(cfg.n_ctiles, 128, cfg.q_half).astype(bf)
        in_maps.append({
            "x": x_bf, "xq": xq_bf,
            "wq": wq_t, "wk": wk_t, "wv": wv_t,
            "wo": wo_t, "bo": bo, "ones": ones, "ind": ind,
        })
    return in_maps


def assemble_output(results, x_shape, cfg=FULL):
    b, dim, H, W = x_shape
    out = np.empty((b, dim, H * W), dtype=np.float32)
    for core, r in enumerate(results):
        bi, half = core // 2, core % 2
        y = r["out"].reshape(dim, cfg.q_half)
        out[bi, :, half * cfg.q_half : (half + 1) * cfg.q_half] = y
    return out.reshape(b, dim, H, W)


_CACHE = {}


def _get_nc():
    if "nc" not in _CACHE:
        _CACHE["nc"] = build_nc()
    return _CACHE["nc"]


def kernel(x, w_qkv, w_out, b_out, trace=False):
    from concourse.bass_utils import run_bass_kernel_spmd

    nc = _get_nc()
    in_maps = make_in_maps(np.asarray(x), np.asarray(w_qkv),
                           np.asarray(w_out), np.asarray(b_out))
    last_err = None
    for _attempt in range(4):
        try:
            res = run_bass_kernel_spmd(nc, in_maps,
                                       core_ids=list(range(N_CORES)),
                                       trace=trace)
            break
        except Exception as e:  # transient NRT device errors
            last_err = e
            res = None
    if res is None:
        raise last_err
    _CACHE["last_result"] = res
    return assemble_output(res.results, np.asarray(x).shape)


# revision 29
# speedup vs baseline: 1.0462x; 1.0462x over previous
"""Distributed Trainium2 Bass kernel for the 1x1-conv multi-head attention block.

Reference computation (per batch b of 4):
    qkv = w_qkv @ x            x: [256, 4096] (channels x spatial), w_qkv: [384, 256]
    q,k,v = split(qkv); per head h (4 heads, d=32): q *= d**-0.5
    sim = q^T k;  attn = softmax(sim, axis=k);  o = attn @ v^T
    y = w_out @ o + b_out      w_out: [256, 128]

Distribution: 8 cores = 4 batches x 2 query-halves. Each core computes
q/k/v projections for its batch (k, v for all 4096 positions; q only for
its 2048 query positions), full attention for its query half across all
4 heads, and the output projection for its rows. No cross-core
reduction is needed; the host concatenates the 8 disjoint output blocks.

On-chip layout ("simT orientation"): scores are computed transposed,
[k_pos (partitions), q_pos (free)], so softmax denominators come from a
ones-vector matmul (partition-dim reduction on the PE) and the attn@v
contraction consumes the exp'd scores directly as the moving operand —
no transposes in the inner loop. exp() has no max-subtraction: scores
are ~N(0,1) so exp is safely in range. All matmuls run in bf16 (PSUM
accumulation is fp32); softmax sums/normalization in fp32.
"""

import sys

if "/opt/trn_rl_repo" not in sys.path:
    sys.path.insert(0, "/opt/trn_rl_repo")

import numpy as np
import ml_dtypes

import concourse.bass as bass
import concourse.mybir as mybir
import concourse.tile as tile
from concourse import bacc

BF16 = mybir.dt.bfloat16
F32 = mybir.dt.float32

N_CORES = 8
HEADS = 4
DIM_HEAD = 32
SCALE = DIM_HEAD ** -0.5


class Cfg:
    def __init__(self, seq=4096, q_half=2048, q_tile=512, dim=256):
        self.seq = seq                  # total k/v positions
        self.q_half = q_half            # q positions handled by this core
        self.q_tile = q_tile            # q columns per PSUM tile (<=512)
        self.dim = dim                  # input channels
        self.hidden = HEADS * DIM_HEAD  # 128
        self.n_ktiles = seq // 128
        self.n_qtiles = q_half // q_tile
        self.n_stiles = seq // q_tile   # spatial tiles for k projection
        self.n_ctiles = dim // 128      # contraction tiles for projections


FULL = Cfg()


def build_nc(cfg=FULL, n_cores=N_CORES, dbg=False):
    nc = bacc.Bacc("TRN2", target_bir_lowering=False, debug=False,
                   num_devices=n_cores)

    # ---- DRAM I/O ----------------------------------------------------
    x_d = nc.dram_tensor("x", [cfg.n_ctiles, 128, cfg.seq], BF16,
                         kind="ExternalInput")
    xq_d = nc.dram_tensor("xq", [cfg.n_ctiles, 128, cfg.q_half], BF16,
                          kind="ExternalInput")
    wq_d = nc.dram_tensor("wq", [cfg.n_ctiles, 128, 128], BF16,
                          kind="ExternalInput")
    wk_d = nc.dram_tensor("wk", [cfg.n_ctiles, 128, 128], BF16,
                          kind="ExternalInput")
    wv_d = nc.dram_tensor("wv", [cfg.n_ctiles, 128, 128], BF16,
                          kind="ExternalInput")
    wo_d = nc.dram_tensor("wo", [128, cfg.dim], BF16, kind="ExternalInput")
    bo_d = nc.dram_tensor("bo", [cfg.n_ctiles, 128, 1], F32, kind="ExternalInput")
    ones_d = nc.dram_tensor("ones", [128, 1], BF16, kind="ExternalInput")
    ind_d = nc.dram_tensor("ind", [HEADS, 128], F32, kind="ExternalInput")
    y_d = nc.dram_tensor("out", [cfg.n_ctiles, 128, cfg.q_half], F32,
                         kind="ExternalOutput")

    dbg_d = None
    if dbg:
        dbg_d = {
            "dbg_qT": nc.dram_tensor("dbg_qT", [128, cfg.q_half], BF16,
                                     kind="ExternalOutput"),
            "dbg_kT": nc.dram_tensor("dbg_kT", [128, cfg.seq], BF16,
                                     kind="ExternalOutput"),
            "dbg_v": nc.dram_tensor("dbg_v", [128, cfg.n_ktiles, HEADS, 33],
                                    BF16, kind="ExternalOutput"),
            "dbg_e": nc.dram_tensor("dbg_e", [128, 2, cfg.q_tile], BF16,
                                    kind="ExternalOutput"),
            "dbg_outn": nc.dram_tensor("dbg_outn", [128, cfg.q_half], BF16,
                                       kind="ExternalOutput"),
        }

    with tile.TileContext(nc) as tc:
        _kernel_body(tc, cfg, x_d, xq_d, wq_d, wk_d, wv_d, wo_d, bo_d,
                     ones_d, ind_d, y_d, dbg_d)
    nc.compile()
    return nc


def _kernel_body(tc, cfg, x_d, xq_d, wq_d, wk_d, wv_d, wo_d, bo_d,
                 ones_d, ind_d, y_d, dbg_d=None):
    nc = tc.nc
    NK, NQ, QT = cfg.n_ktiles, cfg.n_qtiles, cfg.q_tile
    NC_, NS = cfg.n_ctiles, cfg.n_stiles

    from contextlib import ExitStack
    with ExitStack() as ctx:
        ep = ctx.enter_context

        consts = ep(tc.tile_pool(name="consts", bufs=1))
        persist = ep(tc.tile_pool(name="persist", bufs=1))

        # ---- load inputs & weights ----------------------------------
        x_sb = persist.tile([128, NC_, cfg.seq], BF16, tag="x")
        xq_sb = persist.tile([128, NC_, cfg.q_half], BF16, tag="xq")
        wq_sb = consts.tile([128, NC_, 128], BF16, tag="wq")
        wk_sb = consts.tile([128, NC_, 128], BF16, tag="wk")
        wv_sb = consts.tile([128, NC_, 128], BF16, tag="wv")
        wo_sb = consts.tile([128, cfg.dim], BF16, tag="wo")
        bo_sb = consts.tile([128, NC_], F32, tag="bo")
        ones_sb = consts.tile([128, 1], BF16, tag="ones")
        ind_sb = consts.tile([HEADS, 128], F32, tag="ind")

        # Prime the ACT exp table-set before the PE-dense phases: the
        # one-time ~2.7us ACT_TABLE_LOAD otherwise lands on the first
        # real exp, idles the PE long enough for HAM to re-throttle the
        # clock to 1.2 GHz, and it never recovers mid-kernel.
        prime = consts.tile([128, 8], F32, tag="prime")
        nc.vector.memset(prime[:], 0.0)
        nc.scalar.activation(prime[:], prime[:],
                             mybir.ActivationFunctionType.Exp)

        for c in range(NC_):
            nc.sync.dma_start(x_sb[:, c, :], x_d[c])
            nc.sync.dma_start(xq_sb[:, c, :], xq_d[c])
            nc.sync.dma_start(wq_sb[:, c, :], wq_d[c])
            nc.sync.dma_start(wk_sb[:, c, :], wk_d[c])
            nc.sync.dma_start(wv_sb[:, c, :], wv_d[c])
            nc.sync.dma_start(bo_sb[:, c : c + 1], bo_d[c])
        nc.sync.dma_start(wo_sb[:], wo_d[:])
        nc.sync.dma_start(ones_sb[:], ones_d[:])
        nc.sync.dma_start(ind_sb[:], ind_d[:])

        # ---- projections --------------------------------------------
        # qT [d*heads=128, q_half], kT [128, seq] (channel-major)
        qT_sb = persist.tile([128, cfg.q_half], BF16, tag="qT")
        kT_sb = persist.tile([128, cfg.seq], BF16, tag="kT")
        # v spatial-major, augmented with a ones column per head:
        # v_sb[p, t, h, 0:32] = v[t*128+p, h*32+d]; v_sb[p, t, h, 32] = 1
        v_sb = persist.tile([128, NK, HEADS, 33], BF16, tag="v")

        with tc.tile_pool(name="proj_ps", bufs=3, space="PSUM") as proj_ps, \
             tc.tile_pool(name="projv_ps", bufs=3, space="PSUM") as projv_ps:
            # q projection: lhsT = wq tile [128c, 128o], rhs = xq [128c, QT]
            for s in range(cfg.q_half // QT):
                pt = proj_ps.tile([128, QT], F32, tag="proj")
                for c in range(NC_):
                    nc.tensor.matmul(pt[:], wq_sb[:, c, :],
                                     xq_sb[:, c, s * QT : (s + 1) * QT],
                                     start=(c == 0), stop=(c == NC_ - 1))
                nc.vector.tensor_copy(qT_sb[:, s * QT : (s + 1) * QT], pt[:])
            # k projection over full seq
            for s in range(NS):
                pt = proj_ps.tile([128, QT], F32, tag="proj")
                for c in range(NC_):
                    nc.tensor.matmul(pt[:], wk_sb[:, c, :],
                                     x_sb[:, c, s * QT : (s + 1) * QT],
                                     start=(c == 0), stop=(c == NC_ - 1))
                nc.vector.tensor_copy(kT_sb[:, s * QT : (s + 1) * QT], pt[:])
            # v projection, spatial-major: lhsT = x tile [128c, 128s],
            # rhs = wv [128c, 128o] -> out [128s, 128o]
            nc.vector.memset(v_sb[:, :, :, 32], 1.0)
            for t in range(NK):
                pt = projv_ps.tile([128, 128], F32, tag="projv")
                for c in range(NC_):
                    nc.tensor.matmul(pt[:],
                                     x_sb[:, c, t * 128 : (t + 1) * 128],
                                     wv_sb[:, c, :],
                                     start=(c == 0), stop=(c == NC_ - 1))
                for h in range(HEADS):
                    nc.vector.tensor_copy(v_sb[:, t, h, 0:32],
                                          pt[:, 32 * h : 32 * h + 32])

        # ---- attention ----------------------------------------------
        # per q-tile: accumulate outT [128 (h*32+d), QT] and per-head
        # sums [row 32h, QT] over all k tiles.
        attn_ctx = ExitStack()
        with attn_ctx:
            sim_ps = attn_ctx.enter_context(
                tc.tile_pool(name="sim_ps", bufs=2, space="PSUM"))
            acc_ps = attn_ctx.enter_context(
                tc.tile_pool(name="acc_ps", bufs=1, space="PSUM"))
            e_pool = attn_ctx.enter_context(
                tc.tile_pool(name="e_pool", bufs=3))
            norm_pool = attn_ctx.enter_context(
                tc.tile_pool(name="norm", bufs=2))

            outn_sb = persist.tile([128, cfg.q_half], BF16, tag="outn")

            self_attention(nc, cfg, tc, sim_ps, acc_ps, e_pool,
                           norm_pool, qT_sb, kT_sb, v_sb,
                           outn_sb, dbg_d)

        if dbg_d is not None:
            nc.sync.dma_start(dbg_d["dbg_qT"][:], qT_sb[:])
            nc.sync.dma_start(dbg_d["dbg_kT"][:], kT_sb[:])
            nc.sync.dma_start(dbg_d["dbg_v"][:], v_sb[:])
            nc.sync.dma_start(dbg_d["dbg_outn"][:], outn_sb[:])

        # ---- output projection --------------------------------------
        with tc.tile_pool(name="y", bufs=4) as y_pool, \
             tc.tile_pool(name="y_ps", bufs=4, space="PSUM") as yp_ps:
            for c in range(NC_):
                for s in range(cfg.q_half // QT):
                    pt = yp_ps.tile([128, QT], F32, tag="yp")
                    nc.tensor.matmul(pt[:], wo_sb[:, c * 128 : (c + 1) * 128],
                                     outn_sb[:, s * QT : (s + 1) * QT],
                                     start=True, stop=True)
                    yt = y_pool.tile([128, QT], F32, tag="yt")
                    nc.vector.tensor_scalar_add(yt[:], pt[:],
                                                bo_sb[:, c : c + 1])
                    nc.sync.dma_start(y_d[c, :, s * QT : (s + 1) * QT], yt[:])


def self_attention(nc, cfg, tc, sim_ps, acc_ps, e_pool, norm_pool,
                   qT_sb, kT_sb, v_sb, outn_sb, dbg_d=None):
        NK, NQ, QT = cfg.n_ktiles, cfg.n_qtiles, cfg.q_tile
        for qi in range(NQ):
            qs = slice(qi * QT, (qi + 1) * QT)
            accs = [acc_ps.tile([128, QT], F32, tag=f"acc{h}",
                                name=f"acc{h}_{qi}")
                    for h in range(HEADS)]

            for t in range(NK):
                # scores^T for all 4 heads: one 4-wide row-group pack,
                # outputs into two 2-bank psum tiles
                sims = [sim_ps.tile([128, 2, QT], F32, tag="sim",
                                    name=f"sim{qi}_{t}_{p}") for p in range(2)]
                for h in range(HEADS):
                    nc.tensor.matmul(
                        sims[h // 2][:, h % 2, :],
                        kT_sb[32 * h : 32 * h + 32, t * 128 : (t + 1) * 128],
                        qT_sb[32 * h : 32 * h + 32, qs],
                        start=True, stop=True,
                        tile_position=(32 * h, 0),
                    )
                for pair in range(2):
                    e_sb = e_pool.tile([128, 2, QT], BF16, tag="e",
                                       name=f"e{qi}_{t}_{pair}")
                    nc.scalar.activation(e_sb[:], sims[pair][:],
                                         mybir.ActivationFunctionType.Exp)
                    if dbg_d is not None and qi == 0 and t == 0 and pair == 0:
                        nc.sync.dma_start(dbg_d["dbg_e"][:], e_sb[:])
                    for j in range(2):
                        h = pair * 2 + j
                        # attn@v (+ denominator): lhsT = [v_h | 1] chunk
                        # [128 kpos, 33]; row 32 of the output accumulates
                        # sum(E) per q column.
                        nc.tensor.matmul(
                            accs[h][0:33, :],
                            v_sb[:, t, h, :],
                            e_sb[:, j, :],
                            start=(t == 0), stop=(t == NK - 1),
                            skip_group_check=True,
                        )

            # qtile epilogue. Free the acc banks ASAP with one fast copy
            # per head into SBUF scratch; the normalize chain runs off
            # the PE critical path while the next q-tile computes.
            scrs = [norm_pool.tile([33, QT], F32, tag=f"scr{h}",
                                   name=f"scr{h}_{qi}") for h in range(HEADS)]
            for h in range(HEADS):
                nc.vector.tensor_copy(scrs[h][:], accs[h][0:33, :])
            r4 = norm_pool.tile([1, HEADS, QT], F32, tag="r4")
            for h in range(HEADS):
                nc.vector.reciprocal(r4[0:1, h, :], scrs[h][32:33, :])
            bcs = [norm_pool.tile([32, QT], F32, tag=f"bc{h}",
                                  name=f"bc{h}_{qi}")
                   for h in range(HEADS)]
            for h in range(HEADS):
                nc.gpsimd.partition_broadcast(bcs[h][:], r4[0:1, h, :])
            for h in range(HEADS):
                nc.vector.tensor_mul(outn_sb[32 * h : 32 * h + 32, qs],
                                     scrs[h][0:32, :], bcs[h][:])


# ---------------------------------------------------------------------
# host side
# ---------------------------------------------------------------------

def make_in_maps(x, w_qkv, w_out, b_out, cfg=FULL, n_cores=N_CORES):
    """Shard the full inputs into per-core input maps."""
    b, dim, H, W = x.shape
    seq = H * W
    bf = ml_dtypes.bfloat16

    wq = (w_qkv[0:128] * SCALE).astype(np.float32)
    wk = w_qkv[128:256]
    wv = w_qkv[256:384]
    # lhsT layouts [ctile, 128c, 128o]
    wq_t = np.ascontiguousarray(
        wq.T.reshape(cfg.n_ctiles, 128, 128)).astype(bf)
    wk_t = np.ascontiguousarray(
        wk.T.reshape(cfg.n_ctiles, 128, 128)).astype(bf)
    wv_t = np.ascontiguousarray(
        wv.T.reshape(cfg.n_ctiles, 128, 128)).astype(bf)
    wo_t = np.ascontiguousarray(w_out.T).astype(bf)          # [128, 256]
    bo = b_out.reshape(cfg.n_ctiles, 128, 1).astype(np.float32)
    ones = np.ones((128, 1), dtype=bf)
    ind = np.zeros((HEADS, 128), dtype=np.float32)
    for h in range(HEADS):
        ind[h, 32 * h : 32 * h + 32] = 1.0

    in_maps = []
    for core in range(n_cores):
        bi, half = core // 2, core % 2
        xb = x[bi].reshape(dim, seq)
        x_bf = xb.reshape(cfg.n_ctiles, 128, seq).astype(bf)
        xq_bf = np.ascontiguousarray(
            xb[:, half * cfg.q_half : (half + 1) * cfg.q_half]
        ).reshape(cfg.n_ctiles, 128, cfg.q_half).astype(bf)
        in_maps.append({
            "x": x_bf, "xq": xq_bf,
            "wq": wq_t, "wk": wk_t, "wv": wv_t,
            "wo": wo_t, "bo": bo, "ones": ones, "ind": ind,
        })
    return in_maps


def assemble_output(results, x_shape, cfg=FULL):
    b, dim, H, W = x_shape
    out = np.empty((b, dim, H * W), dtype=np.float32)
    for core, r in enumerate(results):
        bi, half = core // 2, core % 2
        y = r["out"].reshape(dim, cfg.q_half)
        out[bi, :, half * cfg.q_half : (half + 1) * cfg.q_half] = y
    return out.reshape(b, dim, H, W)


_CACHE = {}


def _get_nc():
    if "nc" not in _CACHE:
        _CACHE["nc"] = build_nc()
    return _CACHE["nc"]


def kernel(x, w_qkv, w_out, b_out, trace=False):
    from concourse.bass_utils import run_bass_kernel_spmd

    nc = _get_nc()
    in_maps = make_in_maps(np.asarray(x), np.asarray(w_qkv),
                           np.asarray(w_out), np.asarray(b_out))
    last_err = None
    for _attempt in range(4):
        try:
            res = run_bass_kernel_spmd(nc, in_maps,
                                       core_ids=list(range(N_CORES)),
                                       trace=trace)
            break
        except Exception as e:  # transient NRT device errors
            last_err = e
            res = None
    if res is None:
        raise last_err
    _CACHE["last_result"] = res
    return assemble_output(res.results, np.asarray(x).shape)
